# revision 4
# baseline (speedup 1.0000x reference)
"""GCN + DiffPool kernel for Trainium2, data-parallel over graphs across 8 NeuronCores.

Model (per graph, n=150 nodes):
  Z1 = relu(An @ (x @ W1) + b1)          An = D^-1/2 (A+I) D^-1/2
  Z2 = relu(An @ (Z1 @ W2) + b2)
  S  = softmax(An @ (Z2 @ Wa) + ba)      [n, 25]
  Zp = S^T @ Z2 ; Ap = S^T @ (A @ S)
  H  = relu(Anp @ (Zp @ Wp) + bp)        pooled GCN, 25 cluster-nodes
  logits = (sum_rows H) @ Wc + bc

Sharding: 64 graphs -> 8 devices x 8 graphs; block-diagonal adjacency means each
device only gets its 8 graphs' 150x150 blocks (shipped with self-loops
pre-added, i.e. A+I) and its node rows of x (feature-major).

Deferred normalization: An @ M = d .* ((A+I) @ (d .* M)) with d = rsqrt(deg+1).
The row factor is folded into the moving operand (m = d.*M, partition-side
scale, cheap); the column factor d[i'] is applied on the PSUM drain of each
An-matmul (free-side scale against a partition-broadcast dT tile), BEFORE the
per-partition bias + relu of the activation. This lets every An-matmul use the
raw shipped (A+I) tiles directly - no normalized-adjacency build, no
[150,1200] elementwise pass, and the only layout shuffle is a tiny [128,16]
DRAM bounce for dT. A @ S is recovered from (A+I) @ S by subtracting S on the
PSUM drain. colsum+partition-broadcast for the pooled column degrees is fused
into the matmul by using a [*,64] ones block as lhsT. No gpsimd ops anywhere
(SWDGE DMAs and custom-op lib load/unload are expensive).

On-device layout convention:
  fm (feature-major): [feat_part, graph, node]  - used for W-multiplies (lhsT)
  nm (node-major):    [node_part, graph, feat]  - used for A-multiplies
Node dim (150) splits into partition chunks c0=[0:128], c1=[128:150].
"""

import numpy as np

import concourse.bass as bass
import concourse.mybir as mybir
import concourse.tile as tile
from concourse import bacc
from concourse.bass_utils import run_bass_kernel_spmd

F32 = mybir.dt.float32
BF16 = mybir.dt.bfloat16
AF = mybir.ActivationFunctionType
AL = mybir.AluOpType
U32 = mybir.dt.uint32

MMDT = BF16

N_NODES = 9600
N_FEAT = 128
HIDDEN = 64
CLUSTERS = 25
NUM_CLASSES = 10
B_GRAPHS = 64
NPG = 150            # nodes per graph
DEV = 8              # devices
GPD = 8              # graphs per device
C0, C1 = 128, 22     # node partition chunks (128 + 22 = 150)

_CACHE = {}

# wpk (bf16) packed-constant column offsets
WP_W1 = 0                      # [128, 64]
WP_W2 = WP_W1 + HIDDEN         # [64, 64]
WP_WA = WP_W2 + HIDDEN         # [64, 25]
WP_WP = WP_WA + CLUSTERS       # [64, 64]
WP_ONES = WP_WP + HIDDEN       # [128, 64] all-ones block (colsum+bcast lhsT)
WP_ID64 = WP_ONES + HIDDEN     # [64, 64] identity (PE transposes)
WP_COLS = WP_ID64 + HIDDEN

# fpk (f32) packed-constant column offsets
FP_WC = 0                      # [64, 10]
FP_B1 = FP_WC + NUM_CLASSES    # [64, 1]
FP_B2 = FP_B1 + 1
FP_BP = FP_B2 + 1
FP_BC = FP_BP + 1              # [8, 10] bc broadcast over graphs
FP_ID25 = FP_BC + NUM_CLASSES  # [25, 25] identity
FP_BA = FP_ID25 + CLUSTERS     # [128, 25] ba broadcast over partitions
FP_COLS = FP_BA + CLUSTERS


def _chunk(c):
    return (0, C0) if c == 0 else (C0, C1)


def build_nc():
    nc = bacc.Bacc("TRN2", target_bir_lowering=False, debug=False, num_devices=DEV)

    def din(name, shape, dt=F32):
        return nc.dram_tensor(name, shape, dt, kind="ExternalInput").ap()

    ah0 = din("ah0", [C0, GPD, NPG], MMDT)   # (A+I) rows 0:128 per graph
    ah1 = din("ah1", [C1, GPD, NPG], MMDT)   # (A+I) rows 128:150
    xT = din("xT", [N_FEAT, GPD, NPG], MMDT)
    wpk = din("wpk", [N_FEAT, WP_COLS], MMDT)
    fpk = din("fpk", [N_FEAT, FP_COLS], F32)
    out = nc.dram_tensor("out", [GPD, NUM_CLASSES], F32, kind="ExternalOutput").ap()

    with tile.TileContext(nc) as tc:
        with (
            tc.tile_pool(name="cst", bufs=1) as cst,
            tc.tile_pool(name="act", bufs=1) as act,
            tc.tile_pool(name="ps", bufs=7, space="PSUM") as ps,
            tc.tile_pool(name="pst", bufs=1, space="PSUM") as pst,
            tc.tile_pool(name="dram", bufs=1, space="DRAM") as dram,
        ):
            # ---- input DMAs, all HWDGE (sync/scalar). ah0 first: heads the
            # degree->rsqrt critical chain. ---------------------------------
            s_ah0 = cst.tile([C0, GPD, NPG], MMDT, tag="ah0")
            nc.sync.dma_start(out=s_ah0[:], in_=ah0)
            s_ah1 = cst.tile([C1, GPD, NPG], MMDT, tag="ah1")
            nc.scalar.dma_start(out=s_ah1[:], in_=ah1)
            s_wpk = cst.tile([N_FEAT, WP_COLS], MMDT, tag="wpk")
            nc.scalar.dma_start(out=s_wpk[:], in_=wpk)
            s_xT = cst.tile([N_FEAT, GPD, NPG], MMDT, tag="xT")
            nc.sync.dma_start(out=s_xT[:], in_=xT)
            s_fpk = cst.tile([N_FEAT, FP_COLS], F32, tag="fpk")
            nc.scalar.dma_start(out=s_fpk[:], in_=fpk)

            s_ah = (s_ah0, s_ah1)
            s_W1 = s_wpk[:, WP_W1:WP_W1 + HIDDEN]
            s_W2 = s_wpk[0:HIDDEN, WP_W2:WP_W2 + HIDDEN]
            s_Wa = s_wpk[0:HIDDEN, WP_WA:WP_WA + CLUSTERS]
            s_Wp = s_wpk[0:HIDDEN, WP_WP:WP_WP + HIDDEN]
            s_ones = s_wpk[:, WP_ONES:WP_ONES + HIDDEN]
            s_id64 = s_wpk[0:HIDDEN, WP_ID64:WP_ID64 + HIDDEN]
            s_Wc = s_fpk[0:HIDDEN, FP_WC:FP_WC + NUM_CLASSES]
            s_b1 = s_fpk[0:HIDDEN, FP_B1:FP_B1 + 1]
            s_b2 = s_fpk[0:HIDDEN, FP_B2:FP_B2 + 1]
            s_bp = s_fpk[0:HIDDEN, FP_BP:FP_BP + 1]
            s_bc = s_fpk[0:GPD, FP_BC:FP_BC + NUM_CLASSES]
            s_id25 = s_fpk[0:CLUSTERS, FP_ID25:FP_ID25 + CLUSTERS]
            s_baB = s_fpk[:, FP_BA:FP_BA + CLUSTERS]

            # ---- rsqrt helper (quake seed + Newton), all on DVE ------------
            qk1 = act.tile([C0, 1], U32, tag="qk1")
            nc.vector.memset(qk1[:], 1)
            qkm = act.tile([C0, 1], U32, tag="qkm")
            nc.vector.memset(qkm[:], 0x5F3759DF)

            def emit_rsqrt(x, rows, cols, iters=1):
                s = act.tile([rows, cols], F32, tag=f"rs_{id(x)}")
                w = act.tile([rows, cols], F32, tag=f"rw_{id(x)}")
                nc.vector.tensor_tensor(s[:].bitcast(U32), x[:].bitcast(U32),
                                        qk1[0:rows, :].broadcast_to((rows, cols)),
                                        AL.logical_shift_right)
                nc.vector.tensor_tensor(s[:].bitcast(U32),
                                        qkm[0:rows, :].broadcast_to((rows, cols)),
                                        s[:].bitcast(U32), AL.subtract)
                for _ in range(iters):
                    nc.vector.tensor_mul(w[:], s[:], s[:])
                    nc.vector.tensor_mul(w[:], w[:], x[:])
                    nc.vector.tensor_scalar(w[:], w[:], -0.5, 1.5, AL.mult, AL.add)
                    nc.vector.tensor_mul(s[:], s[:], w[:])
                return s

            # ---- d = rsqrt(rowsum(A+I)) in node-major [chunk, graph] -------
            degc = act.tile([C0, 2 * GPD], F32, tag="degc")
            nc.vector.memset(degc[:, GPD:2 * GPD], 1.0)  # unread rows stay finite
            nc.vector.reduce_sum(out=degc[:, 0:GPD], in_=s_ah0[:],
                                 axis=mybir.AxisListType.X)
            nc.vector.reduce_sum(out=degc[0:C1, GPD:2 * GPD], in_=s_ah1[:],
                                 axis=mybir.AxisListType.X)
            dcomb = emit_rsqrt(degc, C0, 2 * GPD, iters=2)
            s_d = [dcomb[:, 0:GPD], dcomb[0:C1, GPD:2 * GPD]]
            dbfc = act.tile([C0, 2 * GPD], MMDT, tag="dbfc")
            nc.vector.tensor_copy(dbfc[:], dcomb[:])

            # ---- dT broadcast via tiny DRAM bounce: [128,16] out, then two
            # rearranged+partition-broadcast reads -> s_dT [64, g, 150] ------
            dd = dram.tile([C0 * 2 * GPD], MMDT, tag="dd")
            nc.sync.dma_start(out=dd[:].rearrange("(j c) -> j c", j=C0), in_=dbfc[:])
            # s_dT[p, j, g] = d[j, g] for all p (node-major free layout keeps
            # every DMA AP <= 3 dims)
            s_dT = cst.tile([HIDDEN, NPG, GPD], MMDT, tag="dT")
            ddv = dd[:].rearrange("(j c) -> j c", j=C0)      # [128, 16] (j, cg)
            src0 = ddv[:, 0:GPD][None, :, :].broadcast_to((HIDDEN, C0, GPD))
            src1 = ddv[0:C1, GPD:2 * GPD][None, :, :].broadcast_to((HIDDEN, C1, GPD))
            nc.sync.dma_start(out=s_dT[:, 0:C0, :], in_=src0)
            nc.scalar.dma_start(out=s_dT[:, C0:NPG, :], in_=src1)

            # ---- helpers ---------------------------------------------------
            def w_mult_nm(lhs_fm, w, kdim, fout, name):
                """m = d .* (Z @ W), node-major chunks. lhsT = fm slice."""
                outs = []
                for c, cn in ((0, C0), (1, C1)):
                    off, _ = _chunk(c)
                    p = ps.tile([cn, GPD, fout], F32, tag="ps")
                    for g in range(GPD):
                        nc.tensor.matmul(p[:, g, :], lhs_fm[0:kdim, g, off:off + cn],
                                         w, start=True, stop=True)
                    o = act.tile([cn, GPD, fout], MMDT, tag=f"{name}{c}")
                    dbc = s_d[c][:][:, :, None].broadcast_to((cn, GPD, fout))
                    nc.vector.tensor_mul(o[:], p[:], dbc)
                    outs.append(o)
                return outs

            def an_mult_fm(m_nm, bias, name):
                """fm out [64, g, 150] = relu(d .* ((A+I) @ m) + bias)."""
                tmp = act.tile([HIDDEN, GPD, NPG], F32, tag=f"{name}t")
                o = act.tile([HIDDEN, GPD, NPG], MMDT, tag=name)
                for g in range(GPD):
                    p = ps.tile([HIDDEN, NPG], F32, tag="ps")
                    nc.tensor.matmul(p[:], m_nm[0][:, g, :], s_ah0[:, g, :],
                                     start=True, stop=False)
                    nc.tensor.matmul(p[:], m_nm[1][:, g, :], s_ah1[:, g, :],
                                     start=False, stop=True)
                    nc.vector.tensor_mul(tmp[:, g, :], p[:], s_dT[:, :, g])
                    nc.scalar.activation(o[:, g, :], tmp[:, g, :], AF.Relu, bias=bias)
                return o

            # ---- encoder ---------------------------------------------------
            m1 = w_mult_nm(s_xT, s_W1, N_FEAT, HIDDEN, "m1")
            z1 = an_mult_fm(m1, s_b1, "z1")
            m2 = w_mult_nm(z1, s_W2, HIDDEN, HIDDEN, "m2")
            z2 = an_mult_fm(m2, s_b2, "z2")

            # ---- Z2 transpose -> nm (for pooling contractions) -------------
            z2n = []
            for c, cn in ((0, C0), (1, C1)):
                off, _ = _chunk(c)
                p = pst.tile([cn, GPD, HIDDEN], MMDT, tag="p2")
                for g in range(GPD):
                    nc.tensor.transpose(p[:, g, :], z2[0:HIDDEN, g, off:off + cn],
                                        s_id64)
                o = act.tile([cn, GPD, HIDDEN], MMDT, tag=f"z2n{c}")
                nc.vector.tensor_copy(o[:], p[:])
                z2n.append(o)

            # ---- assignment: S = softmax(d .* ((A+I) @ v) + ba), nm --------
            v = w_mult_nm(z2, s_Wa, HIDDEN, CLUSTERS, "v")
            s_S = []
            for mc, mn in ((0, C0), (1, C1)):
                moff, _ = _chunk(mc)
                p = ps.tile([mn, GPD, CLUSTERS], F32, tag="ps")
                for g in range(GPD):
                    nc.tensor.matmul(p[:, g, :], s_ah0[:, g, moff:moff + mn],
                                     v[0][:, g, :], start=True, stop=False)
                    nc.tensor.matmul(p[:, g, :], s_ah1[:, g, moff:moff + mn],
                                     v[1][:, g, :], start=False, stop=True)
                dbc = s_d[mc][:][:, :, None].broadcast_to((mn, GPD, CLUSTERS))
                t = act.tile([mn, GPD, CLUSTERS], F32, tag=f"sl{mc}")
                nc.vector.tensor_mul(t[:], p[:], dbc)
                babc = s_baB[0:mn, :][:, None, :].broadcast_to((mn, GPD, CLUSTERS))
                nc.vector.tensor_add(t[:], t[:], babc)
                e = act.tile([mn, GPD, CLUSTERS], F32, tag=f"e{mc}")
                nc.scalar.activation(e[:], t[:], AF.Exp)
                ssum = act.tile([mn, GPD], F32, tag=f"ssum{mc}")
                nc.vector.reduce_sum(out=ssum[:], in_=e[:], axis=mybir.AxisListType.X)
                rs = act.tile([mn, GPD], F32, tag=f"rs{mc}")
                nc.vector.reciprocal(rs[:], ssum[:])
                s = act.tile([mn, GPD, CLUSTERS], MMDT, tag=f"s{mc}")
                nc.vector.tensor_mul(s[:], e[:],
                                     rs[:][:, :, None].broadcast_to((mn, GPD, CLUSTERS)))
                s_S.append(s)

            # ---- AS = A @ S = (A+I) @ S - S, nm ----------------------------
            s_AS = []
            for mc, mn in ((0, C0), (1, C1)):
                moff, _ = _chunk(mc)
                p = ps.tile([mn, GPD, CLUSTERS], F32, tag="ps")
                for g in range(GPD):
                    nc.tensor.matmul(p[:, g, :], s_ah0[:, g, moff:moff + mn],
                                     s_S[0][:, g, :], start=True, stop=False)
                    nc.tensor.matmul(p[:, g, :], s_ah1[:, g, moff:moff + mn],
                                     s_S[1][:, g, :], start=False, stop=True)
                o = act.tile([mn, GPD, CLUSTERS], MMDT, tag=f"as{mc}")
                nc.vector.tensor_tensor(o[:], p[:], s_S[mc][:], AL.subtract)
                s_AS.append(o)

            # ---- pooled column degrees: colsum(AS) broadcast to 64 rows by
            # using a ones-block lhsT; +1 then rsqrt -> dpT [64, (g,25)] -----
            p_cs = pst.tile([HIDDEN, GPD * CLUSTERS], F32, tag="p2")
            as0f = s_AS[0][:].rearrange("p g c -> p (g c)")
            as1f = s_AS[1][:].rearrange("p g c -> p (g c)")
            nc.tensor.matmul(p_cs[:], s_ones[0:C0, :], as0f, start=True, stop=False)
            nc.tensor.matmul(p_cs[:], s_ones[0:C1, :], as1f, start=False, stop=True)
            ubc = act.tile([HIDDEN, GPD * CLUSTERS], F32, tag="ubc")
            nc.vector.tensor_scalar_add(ubc[:], p_cs[:], 1.0)
            dpT2 = emit_rsqrt(ubc, HIDDEN, GPD * CLUSTERS, iters=1)
            s_dpT = dpT2[:].rearrange("p (g j) -> p g j", g=GPD)

            # ---- Ap = S^T @ AS (PSUM), row degrees + dp --------------------
            p_ap = ps.tile([CLUSTERS, GPD, CLUSTERS], F32, tag="ps")
            for g in range(GPD):
                nc.tensor.matmul(p_ap[:, g, :], s_S[0][:, g, :], s_AS[0][:, g, :],
                                 start=True, stop=False)
                nc.tensor.matmul(p_ap[:, g, :], s_S[1][:, g, :], s_AS[1][:, g, :],
                                 start=False, stop=True)
            degp = act.tile([CLUSTERS, GPD], F32, tag="degp")
            nc.vector.reduce_sum(out=degp[:], in_=p_ap[:], axis=mybir.AxisListType.X)
            nc.vector.tensor_scalar_add(degp[:], degp[:], 1.0)
            dp = emit_rsqrt(degp, CLUSTERS, GPD, iters=1)

            # ---- Zp = S^T @ Z2, fm [64, g, 25] -----------------------------
            p_zp = ps.tile([HIDDEN, GPD, CLUSTERS], F32, tag="ps")
            for g in range(GPD):
                nc.tensor.matmul(p_zp[:, g, :], z2n[0][:, g, :], s_S[0][:, g, :],
                                 start=True, stop=False)
                nc.tensor.matmul(p_zp[:, g, :], z2n[1][:, g, :], s_S[1][:, g, :],
                                 start=False, stop=True)
            s_Zp = act.tile([HIDDEN, GPD, CLUSTERS], MMDT, tag="zp")
            nc.scalar.copy(s_Zp[:], p_zp[:])

            # ---- ahp = Ap + I (raw, normalization deferred) ----------------
            ahp = act.tile([CLUSTERS, GPD, CLUSTERS], MMDT, tag="ahp")
            id25b = s_id25[:, None, :].broadcast_to((CLUSTERS, GPD, CLUSTERS))
            nc.vector.tensor_add(ahp[:], p_ap[:], id25b)

            # ---- pooled GCN: H = relu(dp' .* ((Ap+I) @ (dp .* ZpWp)) + bp) -
            p_zw = ps.tile([CLUSTERS, GPD, HIDDEN], F32, tag="ps")
            for g in range(GPD):
                nc.tensor.matmul(p_zw[:, g, :], s_Zp[:, g, :], s_Wp,
                                 start=True, stop=True)
            mp = act.tile([CLUSTERS, GPD, HIDDEN], MMDT, tag="mp")
            nc.vector.tensor_mul(mp[:], p_zw[:],
                                 dp[:][:, :, None].broadcast_to((CLUSTERS, GPD, HIDDEN)))

            p_h = ps.tile([HIDDEN, GPD, CLUSTERS], F32, tag="ps")
            for g in range(GPD):
                nc.tensor.matmul(p_h[:, g, :], mp[:, g, :], ahp[:, g, :],
                                 start=True, stop=True)
            th = act.tile([HIDDEN, GPD, CLUSTERS], F32, tag="th")
            nc.vector.tensor_mul(th[:], p_h[:], s_dpT)
            s_H = act.tile([HIDDEN, GPD, CLUSTERS], F32, tag="h")
            nc.scalar.activation(s_H[:], th[:], AF.Relu, bias=s_bp)

            # ---- readout + classifier --------------------------------------
            s_G = act.tile([HIDDEN, GPD], F32, tag="g")
            nc.vector.reduce_sum(out=s_G[:], in_=s_H[:], axis=mybir.AxisListType.X)

            p_l = ps.tile([GPD, NUM_CLASSES], F32, tag="ps")
            nc.tensor.matmul(p_l[:], s_G[:], s_Wc, start=True, stop=True)
            s_out = act.tile([GPD, NUM_CLASSES], F32, tag="logits")
            nc.vector.tensor_add(s_out[:], p_l[:], s_bc)
            nc.sync.dma_start(out=out, in_=s_out[:])

    nc.compile()
    return nc


def make_in_maps(x, a, W1, b1, W2, b2, Wa, ba, Wp, bp, Wc, bc):
    import ml_dtypes
    npmm = np.dtype(ml_dtypes.bfloat16) if MMDT == BF16 else np.dtype(np.float32)

    x = np.ascontiguousarray(np.asarray(x, dtype=np.float32))
    a = np.asarray(a, dtype=np.float32)

    # diagonal 150x150 blocks of the batch adjacency, self-loops pre-added
    ab = a.reshape(B_GRAPHS, NPG, B_GRAPHS, NPG)
    blocks = ab[np.arange(B_GRAPHS), :, np.arange(B_GRAPHS), :]  # [64, 150, 150]
    blocks = blocks + np.eye(NPG, dtype=np.float32)[None]
    blocks = blocks.astype(npmm)

    wpk = np.zeros((N_FEAT, WP_COLS), npmm)
    wpk[:, WP_W1:WP_W1 + HIDDEN] = np.asarray(W1, np.float32).astype(npmm)
    wpk[0:HIDDEN, WP_W2:WP_W2 + HIDDEN] = np.asarray(W2, np.float32).astype(npmm)
    wpk[0:HIDDEN, WP_WA:WP_WA + CLUSTERS] = np.asarray(Wa, np.float32).astype(npmm)
    wpk[0:HIDDEN, WP_WP:WP_WP + HIDDEN] = np.asarray(Wp, np.float32).astype(npmm)
    wpk[:, WP_ONES:WP_ONES + HIDDEN] = 1.0
    wpk[0:HIDDEN, WP_ID64:WP_ID64 + HIDDEN] = np.eye(HIDDEN, dtype=npmm)

    fpk = np.zeros((N_FEAT, FP_COLS), np.float32)
    fpk[0:HIDDEN, FP_WC:FP_WC + NUM_CLASSES] = np.asarray(Wc, np.float32)
    fpk[0:HIDDEN, FP_B1] = np.asarray(b1, np.float32)
    fpk[0:HIDDEN, FP_B2] = np.asarray(b2, np.float32)
    fpk[0:HIDDEN, FP_BP] = np.asarray(bp, np.float32)
    fpk[0:GPD, FP_BC:FP_BC + NUM_CLASSES] = np.asarray(bc, np.float32)[None, :]
    fpk[0:CLUSTERS, FP_ID25:FP_ID25 + CLUSTERS] = np.eye(CLUSTERS, dtype=np.float32)
    fpk[:, FP_BA:FP_BA + CLUSTERS] = np.asarray(ba, np.float32)[None, :]

    common = dict(wpk=wpk, fpk=fpk)

    in_maps = []
    for d in range(DEV):
        xd = x[d * GPD * NPG:(d + 1) * GPD * NPG]          # [1200, 128]
        xT = np.ascontiguousarray(xd.T).reshape(N_FEAT, GPD, NPG).astype(npmm)
        bd = blocks[d * GPD:(d + 1) * GPD]                  # [8, 150, 150]
        bt = np.ascontiguousarray(bd.transpose(1, 0, 2))    # [150, 8, 150]
        in_maps.append(dict(
            xT=xT,
            ah0=np.ascontiguousarray(bt[:C0]),
            ah1=np.ascontiguousarray(bt[C0:]),
            **common,
        ))
    return in_maps


def kernel(x, a, seg_ids, num_graphs, W1, b1, W2, b2, Wa, ba, Wp, bp, Wc, bc,
           trace=False):
    if "nc" not in _CACHE:
        _CACHE["nc"] = build_nc()
    nc = _CACHE["nc"]
    in_maps = make_in_maps(x, a, W1, b1, W2, b2, Wa, ba, Wp, bp, Wc, bc)
    res = run_bass_kernel_spmd(nc, in_maps, core_ids=list(range(DEV)), trace=trace)
    logits = np.concatenate([r["out"] for r in res.results], axis=0)
    if trace:
        return logits, res
    return logits


# revision 13
# speedup vs baseline: 1.1018x; 1.1018x over previous
"""GCN + DiffPool kernel for Trainium2, data-parallel over graphs across 8 NeuronCores.

Model (per graph, n=150 nodes):
  Z1 = relu(An @ (x @ W1) + b1)          An = D^-1/2 (A+I) D^-1/2
  Z2 = relu(An @ (Z1 @ W2) + b2)
  S  = softmax(An @ (Z2 @ Wa) + ba)      [n, 25]
  Zp = S^T @ Z2 ; Ap = S^T @ (A @ S)
  H  = relu(Anp @ (Zp @ Wp) + bp)        pooled GCN, 25 cluster-nodes
  logits = (sum_rows H) @ Wc + bc

Sharding: 64 graphs -> 8 devices x 8 graphs; block-diagonal adjacency means each
device only gets its 8 graphs' 150x150 blocks (shipped with self-loops
pre-added, i.e. A+I) and its node rows of x (feature-major).

Deferred normalization: An @ M = d .* ((A+I) @ (d .* M)) with d = rsqrt(deg+1).
The row factor is folded into the moving operand (m = d.*M, partition-side
scale, cheap); the column factor d[i'] is applied on the PSUM drain of each
An-matmul (free-side scale against a partition-broadcast dT tile), BEFORE the
per-partition bias + relu of the activation. This lets every An-matmul use the
raw shipped (A+I) tiles directly - no normalized-adjacency build, no
[150,1200] elementwise pass, and the only layout shuffle is a tiny [128,16]
DRAM bounce for dT. A @ S is recovered from (A+I) @ S by subtracting S on the
PSUM drain. colsum+partition-broadcast for the pooled column degrees is fused
into the matmul by using a [*,64] ones block as lhsT. No gpsimd ops anywhere
(SWDGE DMAs and custom-op lib load/unload are expensive).

On-device layout convention:
  fm (feature-major): [feat_part, graph, node]  - used for W-multiplies (lhsT)
  nm (node-major):    [node_part, graph, feat]  - used for A-multiplies
Node dim (150) splits into partition chunks c0=[0:128], c1=[128:150].
"""

import numpy as np

import concourse.bass as bass
import concourse.mybir as mybir
import concourse.tile as tile
from concourse import bacc
from concourse.bass_utils import run_bass_kernel_spmd

F32 = mybir.dt.float32
BF16 = mybir.dt.bfloat16
AF = mybir.ActivationFunctionType
AL = mybir.AluOpType
U32 = mybir.dt.uint32

MMDT = BF16

N_NODES = 9600
N_FEAT = 128
HIDDEN = 64
CLUSTERS = 25
NUM_CLASSES = 10
B_GRAPHS = 64
NPG = 150            # nodes per graph
DEV = 8              # devices
GPD = 8              # graphs per device
C0, C1 = 128, 22     # node partition chunks (128 + 22 = 150)

_CACHE = {}

# wpk (bf16) packed-constant column offsets
WP_W1 = 0                      # [128, 64]
WP_W2 = WP_W1 + HIDDEN         # [64, 64]
WP_WA = WP_W2 + HIDDEN         # [64, 25]
WP_WP = WP_WA + CLUSTERS       # [64, 64]
WP_ONES = WP_WP + HIDDEN       # [128, 64] all-ones block (colsum+bcast lhsT)
WP_ID64 = WP_ONES + HIDDEN     # [64, 64] identity (z2 transposes)
WP_ID128 = WP_ID64 + HIDDEN    # [128, 128] identity (d transposes)
WP_COLS = WP_ID128 + N_FEAT

# fpk (f32) packed-constant column offsets
FP_WC = 0                      # [64, 10]
FP_B1 = FP_WC + NUM_CLASSES    # [64, 1]
FP_B2 = FP_B1 + 1
FP_BP = FP_B2 + 1
FP_BC = FP_BP + 1              # [8, 10] bc broadcast over graphs
FP_ID25 = FP_BC + NUM_CLASSES  # [25, 25] identity
FP_EBA = FP_ID25 + CLUSTERS    # [128, 25] exp(ba) broadcast over partitions
FP_COLS = FP_EBA + CLUSTERS


def _chunk(c):
    return (0, C0) if c == 0 else (C0, C1)


def build_nc():
    nc = bacc.Bacc("TRN2", target_bir_lowering=False, debug=False, num_devices=DEV)

    def din(name, shape, dt=F32):
        return nc.dram_tensor(name, shape, dt, kind="ExternalInput").ap()

    ah0 = din("ah0", [C0, GPD, NPG], MMDT)   # (A+I) rows 0:128 per graph
    ah1 = din("ah1", [C1, GPD, NPG], MMDT)   # (A+I) rows 128:150
    xT = din("xT", [N_FEAT, GPD, NPG], MMDT)
    wpk = din("wpk", [N_FEAT, WP_COLS], MMDT)
    fpk = din("fpk", [N_FEAT, FP_COLS], F32)
    out = nc.dram_tensor("out", [GPD, NUM_CLASSES], F32, kind="ExternalOutput").ap()

    with tile.TileContext(nc) as tc:
        with (
            tc.tile_pool(name="cst", bufs=1) as cst,
            tc.tile_pool(name="act", bufs=1) as act,
            tc.tile_pool(name="ps", bufs=7, space="PSUM") as ps,
            tc.tile_pool(name="pst", bufs=1, space="PSUM") as pst,
            tc.tile_pool(name="dram", bufs=1, space="DRAM") as dram,
        ):
            # ---- input DMAs, all HWDGE (sync/scalar). Adjacency first: it
            # heads the degree->rsqrt->An critical chain. -------------------
            HG = GPD // 2
            s_ah0 = cst.tile([C0, GPD, NPG], MMDT, tag="ah0")
            nc.sync.dma_start(out=s_ah0[:, 0:HG, :], in_=ah0[:, 0:HG, :])
            s_ah1 = cst.tile([C1, GPD, NPG], MMDT, tag="ah1")
            nc.scalar.dma_start(out=s_ah1[:], in_=ah1)
            nc.scalar.dma_start(out=s_ah0[:, HG:GPD, :], in_=ah0[:, HG:GPD, :])
            s_xT = cst.tile([N_FEAT, GPD, NPG], MMDT, tag="xT")
            nc.sync.dma_start(out=s_xT[:], in_=xT)
            s_wpk = cst.tile([N_FEAT, WP_COLS], MMDT, tag="wpk")
            nc.scalar.dma_start(out=s_wpk[:], in_=wpk)
            s_fpk = cst.tile([N_FEAT, FP_COLS], F32, tag="fpk")
            nc.scalar.dma_start(out=s_fpk[:], in_=fpk)

            s_W1 = s_wpk[:, WP_W1:WP_W1 + HIDDEN]
            s_W2 = s_wpk[0:HIDDEN, WP_W2:WP_W2 + HIDDEN]
            s_Wa = s_wpk[0:HIDDEN, WP_WA:WP_WA + CLUSTERS]
            s_Wp = s_wpk[0:HIDDEN, WP_WP:WP_WP + HIDDEN]
            s_ones = s_wpk[:, WP_ONES:WP_ONES + HIDDEN]
            s_id64 = s_wpk[0:HIDDEN, WP_ID64:WP_ID64 + HIDDEN]
            s_id128 = s_wpk[:, WP_ID128:WP_ID128 + N_FEAT]
            s_Wc = s_fpk[0:HIDDEN, FP_WC:FP_WC + NUM_CLASSES]
            s_b1 = s_fpk[0:HIDDEN, FP_B1:FP_B1 + 1]
            s_b2 = s_fpk[0:HIDDEN, FP_B2:FP_B2 + 1]
            s_bp = s_fpk[0:HIDDEN, FP_BP:FP_BP + 1]
            s_bc = s_fpk[0:GPD, FP_BC:FP_BC + NUM_CLASSES]
            s_id25 = s_fpk[0:CLUSTERS, FP_ID25:FP_ID25 + CLUSTERS]
            s_ebaB = s_fpk[:, FP_EBA:FP_EBA + CLUSTERS]

            # ---- rsqrt helper (quake seed + Newton), all on DVE ------------
            qk1 = act.tile([C0, 1], U32, tag="qk1")
            nc.vector.memset(qk1[:], 1)
            qkm = act.tile([C0, 1], U32, tag="qkm")
            nc.vector.memset(qkm[:], 0x5F3759DF)

            def emit_rsqrt(x, rows, cols, iters=1):
                s = act.tile([rows, cols], F32, tag=f"rs_{id(x)}")
                w = act.tile([rows, cols], F32, tag=f"rw_{id(x)}")
                nc.vector.tensor_tensor(s[:].bitcast(U32), x[:].bitcast(U32),
                                        qk1[0:rows, :].broadcast_to((rows, cols)),
                                        AL.logical_shift_right)
                nc.vector.tensor_tensor(s[:].bitcast(U32),
                                        qkm[0:rows, :].broadcast_to((rows, cols)),
                                        s[:].bitcast(U32), AL.subtract)
                for _ in range(iters):
                    nc.vector.tensor_mul(w[:], s[:], s[:])
                    nc.vector.tensor_mul(w[:], w[:], x[:])
                    nc.vector.tensor_scalar(w[:], w[:], -0.5, 1.5, AL.mult, AL.add)
                    nc.vector.tensor_mul(s[:], s[:], w[:])
                return s

            # ---- d = rsqrt(rowsum(A+I)) in node-major [chunk, graph].
            # a1 reduce first (its DMA lands first); a0 in halves. ----------
            degc = act.tile([C0, 2 * GPD], F32, tag="degc")
            nc.vector.memset(degc[:, GPD:2 * GPD], 1.0)  # unread rows stay finite
            nc.vector.reduce_sum(out=degc[0:C1, GPD:2 * GPD], in_=s_ah1[:],
                                 axis=mybir.AxisListType.X)
            nc.vector.reduce_sum(out=degc[:, 0:HG], in_=s_ah0[:, 0:HG, :],
                                 axis=mybir.AxisListType.X)
            nc.vector.reduce_sum(out=degc[:, HG:GPD], in_=s_ah0[:, HG:GPD, :],
                                 axis=mybir.AxisListType.X)
            dcomb = emit_rsqrt(degc, C0, 2 * GPD, iters=1)
            s_d = [dcomb[:, 0:GPD], dcomb[0:C1, GPD:2 * GPD]]
            dbfc = act.tile([C0, 2 * GPD], MMDT, tag="dbfc")
            nc.vector.tensor_copy(dbfc[:], dcomb[:])

            # ---- dT broadcast: PE-transpose d to [8,150], bounce through a
            # 2.4KB DRAM row, read back with a partition-broadcast AP -------
            p_dt = pst.tile([GPD, NPG], MMDT, tag="p2")
            nc.tensor.transpose(p_dt[:, 0:C0], dbfc[:, 0:GPD], s_id128)
            nc.tensor.transpose(p_dt[:, C0:NPG], dbfc[0:C1, GPD:2 * GPD],
                                s_id128[0:C1, 0:C1])
            dTrow = act.tile([GPD, NPG], MMDT, tag="dTrow")
            nc.vector.tensor_copy(dTrow[:], p_dt[:])
            dd = dram.tile([GPD * NPG], MMDT, tag="dd")
            nc.sync.dma_start(out=dd[:].rearrange("(g j) -> g j", g=GPD),
                              in_=dTrow[:])
            s_dT = cst.tile([C0, GPD, NPG], MMDT, tag="dT")
            dsrc = dd[:].rearrange("(g j) -> g j", g=GPD)[None, :, :] \
                .broadcast_to((C0, GPD, NPG))
            nc.sync.dma_start(out=s_dT[:, 0:HG, :], in_=dsrc[:, 0:HG, :])
            nc.scalar.dma_start(out=s_dT[:, HG:GPD, :], in_=dsrc[:, HG:GPD, :])

            # ---- helpers ---------------------------------------------------
            def w_mult_nm(lhs_fm, w, kdim, fout, name):
                """m = d .* (Z @ W), node-major chunks. lhsT = fm slice."""
                outs = []
                for c, cn in ((0, C0), (1, C1)):
                    off, _ = _chunk(c)
                    p = ps.tile([cn, GPD, fout], F32, tag="ps")
                    for g in range(GPD):
                        nc.tensor.matmul(p[:, g, :], lhs_fm[0:kdim, g, off:off + cn],
                                         w, start=True, stop=True)
                    o = act.tile([cn, GPD, fout], MMDT, tag=f"{name}{c}")
                    dbc = s_d[c][:][:, :, None].broadcast_to((cn, GPD, fout))
                    nc.vector.tensor_mul(o[:], p[:], dbc)
                    outs.append(o)
                return outs

            # m1 = d .* (x @ W1) BEFORE the An build so its DVE drain runs
            # while the dT bounce DMAs are in flight.
            m1 = w_mult_nm(s_xT, s_W1, N_FEAT, HIDDEN, "m1")

            # ---- An_col = (A+I) .* dT_bc, interleaved g-halves so z1 can
            # start on the first graphs early -------------------------------
            an0 = act.tile([C0, GPD, NPG], MMDT, tag="an0")
            an1 = act.tile([C1, GPD, NPG], MMDT, tag="an1")
            s_an = (an0, an1)
            for h in range(2):
                gl, gh = h * HG, (h + 1) * HG
                nc.vector.tensor_mul(an0[:, gl:gh, :], s_ah0[:, gl:gh, :],
                                     s_dT[:, gl:gh, :])
                nc.vector.tensor_mul(an1[:, gl:gh, :], s_ah1[:, gl:gh, :],
                                     s_dT[0:C1, gl:gh, :])

            def an_mult_fm(m_nm, bias, name):
                """fm out [64, g, 150] = relu(An @ m + bias), graph-paired
                PSUM tiles, ACT-only drain."""
                o = act.tile([HIDDEN, GPD, NPG], MMDT, tag=name)
                for q in range(GPD // 2):
                    p = ps.tile([HIDDEN, 2, NPG], F32, tag="ps")
                    for k in range(2):
                        g = 2 * q + k
                        nc.tensor.matmul(p[:, k, :], m_nm[0][:, g, :],
                                         an0[:, g, :], start=True, stop=False)
                        nc.tensor.matmul(p[:, k, :], m_nm[1][:, g, :],
                                         an1[:, g, :], start=False, stop=True)
                    nc.scalar.activation(o[:, 2 * q:2 * q + 2, :], p[:],
                                         AF.Relu, bias=bias)
                return o

            # ---- encoder ---------------------------------------------------
            z1 = an_mult_fm(m1, s_b1, "z1")
            m2 = w_mult_nm(z1, s_W2, HIDDEN, HIDDEN, "m2")
            z2 = an_mult_fm(m2, s_b2, "z2")

            # ---- Z2 transpose -> nm (for pooling contractions) -------------
            z2n = []
            for c, cn in ((0, C0), (1, C1)):
                off, _ = _chunk(c)
                p = pst.tile([cn, GPD, HIDDEN], MMDT, tag="p2")
                for g in range(GPD):
                    nc.tensor.transpose(p[:, g, :], z2[0:HIDDEN, g, off:off + cn],
                                        s_id64)
                o = act.tile([cn, GPD, HIDDEN], MMDT, tag=f"z2n{c}")
                nc.scalar.copy(o[:], p[:])
                z2n.append(o)

            # ---- assignment: S = softmax(An @ v + ba), nm.  ba enters as a
            # host-precomputed exp(ba) factor on the softmax numerator. -----
            v = w_mult_nm(z2, s_Wa, HIDDEN, CLUSTERS, "v")
            s_S = []
            for mc, mn in ((0, C0), (1, C1)):
                moff, _ = _chunk(mc)
                p = ps.tile([mn, GPD, CLUSTERS], F32, tag="ps")
                for g in range(GPD):
                    nc.tensor.matmul(p[:, g, :], an0[:, g, moff:moff + mn],
                                     v[0][:, g, :], start=True, stop=False)
                    nc.tensor.matmul(p[:, g, :], an1[:, g, moff:moff + mn],
                                     v[1][:, g, :], start=False, stop=True)
                e = act.tile([mn, GPD, CLUSTERS], F32, tag=f"e{mc}")
                nc.scalar.activation(e[:], p[:], AF.Exp)
                ebabc = s_ebaB[0:mn, :][:, None, :].broadcast_to((mn, GPD, CLUSTERS))
                nc.vector.tensor_mul(e[:], e[:], ebabc)
                ssum = act.tile([mn, GPD], F32, tag=f"ssum{mc}")
                nc.vector.reduce_sum(out=ssum[:], in_=e[:], axis=mybir.AxisListType.X)
                rs = act.tile([mn, GPD], F32, tag=f"rs{mc}")
                nc.vector.reciprocal(rs[:], ssum[:])
                s = act.tile([mn, GPD, CLUSTERS], MMDT, tag=f"s{mc}")
                nc.vector.tensor_mul(s[:], e[:],
                                     rs[:][:, :, None].broadcast_to((mn, GPD, CLUSTERS)))
                s_S.append(s)

            # ---- AS = A @ S = (A+I) @ S - S, nm ----------------------------
            s_AS = []
            for mc, mn in ((0, C0), (1, C1)):
                moff, _ = _chunk(mc)
                p = ps.tile([mn, GPD, CLUSTERS], F32, tag="ps")
                for g in range(GPD):
                    nc.tensor.matmul(p[:, g, :], s_ah0[:, g, moff:moff + mn],
                                     s_S[0][:, g, :], start=True, stop=False)
                    nc.tensor.matmul(p[:, g, :], s_ah1[:, g, moff:moff + mn],
                                     s_S[1][:, g, :], start=False, stop=True)
                o = act.tile([mn, GPD, CLUSTERS], MMDT, tag=f"as{mc}")
                nc.vector.tensor_tensor(o[:], p[:], s_S[mc][:], AL.subtract)
                s_AS.append(o)

            # ---- pooled column degrees: colsum(AS) broadcast to 64 rows by
            # using a ones-block lhsT; +1 then rsqrt -> dpT [64, (g,25)] -----
            p_cs = pst.tile([HIDDEN, GPD * CLUSTERS], F32, tag="p2")
            as0f = s_AS[0][:].rearrange("p g c -> p (g c)")
            as1f = s_AS[1][:].rearrange("p g c -> p (g c)")
            nc.tensor.matmul(p_cs[:], s_ones[0:C0, :], as0f, start=True, stop=False)
            nc.tensor.matmul(p_cs[:], s_ones[0:C1, :], as1f, start=False, stop=True)
            ubc = act.tile([HIDDEN, GPD * CLUSTERS], F32, tag="ubc")
            nc.vector.tensor_scalar_add(ubc[:], p_cs[:], 1.0)
            dpT2 = emit_rsqrt(ubc, HIDDEN, GPD * CLUSTERS, iters=1)
            s_dpT = dpT2[:].rearrange("p (g j) -> p g j", g=GPD)

            # ---- Ap = S^T @ AS (PSUM), row degrees + dp --------------------
            p_ap = ps.tile([CLUSTERS, GPD, CLUSTERS], F32, tag="ps")
            for g in range(GPD):
                nc.tensor.matmul(p_ap[:, g, :], s_S[0][:, g, :], s_AS[0][:, g, :],
                                 start=True, stop=False)
                nc.tensor.matmul(p_ap[:, g, :], s_S[1][:, g, :], s_AS[1][:, g, :],
                                 start=False, stop=True)
            degp = act.tile([CLUSTERS, GPD], F32, tag="degp")
            nc.vector.reduce_sum(out=degp[:], in_=p_ap[:], axis=mybir.AxisListType.X)
            nc.vector.tensor_scalar_add(degp[:], degp[:], 1.0)
            dp = emit_rsqrt(degp, CLUSTERS, GPD, iters=1)

            # ---- Zp = S^T @ Z2, fm [64, g, 25] -----------------------------
            p_zp = ps.tile([HIDDEN, GPD, CLUSTERS], F32, tag="ps")
            for g in range(GPD):
                nc.tensor.matmul(p_zp[:, g, :], z2n[0][:, g, :], s_S[0][:, g, :],
                                 start=True, stop=False)
                nc.tensor.matmul(p_zp[:, g, :], z2n[1][:, g, :], s_S[1][:, g, :],
                                 start=False, stop=True)
            s_Zp = act.tile([HIDDEN, GPD, CLUSTERS], MMDT, tag="zp")
            nc.scalar.copy(s_Zp[:], p_zp[:])

            # ---- ahp = Ap + I (raw, normalization deferred) ----------------
            ahp = act.tile([CLUSTERS, GPD, CLUSTERS], MMDT, tag="ahp")
            id25b = s_id25[:, None, :].broadcast_to((CLUSTERS, GPD, CLUSTERS))
            nc.vector.tensor_add(ahp[:], p_ap[:], id25b)

            # ---- pooled GCN: H = relu(dp' .* ((Ap+I) @ (dp .* ZpWp)) + bp) -
            p_zw = ps.tile([CLUSTERS, GPD, HIDDEN], F32, tag="ps")
            for g in range(GPD):
                nc.tensor.matmul(p_zw[:, g, :], s_Zp[:, g, :], s_Wp,
                                 start=True, stop=True)
            mp = act.tile([CLUSTERS, GPD, HIDDEN], MMDT, tag="mp")
            nc.vector.tensor_mul(mp[:], p_zw[:],
                                 dp[:][:, :, None].broadcast_to((CLUSTERS, GPD, HIDDEN)))

            p_h = ps.tile([HIDDEN, GPD, CLUSTERS], F32, tag="ps")
            for g in range(GPD):
                nc.tensor.matmul(p_h[:, g, :], mp[:, g, :], ahp[:, g, :],
                                 start=True, stop=True)
            th = act.tile([HIDDEN, GPD, CLUSTERS], F32, tag="th")
            nc.vector.tensor_mul(th[:], p_h[:], s_dpT)
            s_H = act.tile([HIDDEN, GPD, CLUSTERS], F32, tag="h")
            nc.scalar.activation(s_H[:], th[:], AF.Relu, bias=s_bp)

            # ---- readout + classifier --------------------------------------
            s_G = act.tile([HIDDEN, GPD], F32, tag="g")
            nc.vector.reduce_sum(out=s_G[:], in_=s_H[:], axis=mybir.AxisListType.X)

            p_l = ps.tile([GPD, NUM_CLASSES], F32, tag="ps")
            nc.tensor.matmul(p_l[:], s_G[:], s_Wc, start=True, stop=True)
            s_out = act.tile([GPD, NUM_CLASSES], F32, tag="logits")
            nc.vector.tensor_add(s_out[:], p_l[:], s_bc)
            nc.sync.dma_start(out=out, in_=s_out[:])

    nc.compile()
    return nc


def make_in_maps(x, a, W1, b1, W2, b2, Wa, ba, Wp, bp, Wc, bc):
    import ml_dtypes
    npmm = np.dtype(ml_dtypes.bfloat16) if MMDT == BF16 else np.dtype(np.float32)

    x = np.ascontiguousarray(np.asarray(x, dtype=np.float32))
    a = np.asarray(a, dtype=np.float32)

    # diagonal 150x150 blocks of the batch adjacency, self-loops pre-added
    ab = a.reshape(B_GRAPHS, NPG, B_GRAPHS, NPG)
    blocks = ab[np.arange(B_GRAPHS), :, np.arange(B_GRAPHS), :]  # [64, 150, 150]
    blocks = blocks + np.eye(NPG, dtype=np.float32)[None]
    blocks = blocks.astype(npmm)

    wpk = np.zeros((N_FEAT, WP_COLS), npmm)
    wpk[:, WP_W1:WP_W1 + HIDDEN] = np.asarray(W1, np.float32).astype(npmm)
    wpk[0:HIDDEN, WP_W2:WP_W2 + HIDDEN] = np.asarray(W2, np.float32).astype(npmm)
    wpk[0:HIDDEN, WP_WA:WP_WA + CLUSTERS] = np.asarray(Wa, np.float32).astype(npmm)
    wpk[0:HIDDEN, WP_WP:WP_WP + HIDDEN] = np.asarray(Wp, np.float32).astype(npmm)
    wpk[:, WP_ONES:WP_ONES + HIDDEN] = 1.0
    wpk[0:HIDDEN, WP_ID64:WP_ID64 + HIDDEN] = np.eye(HIDDEN, dtype=npmm)
    wpk[:, WP_ID128:WP_ID128 + N_FEAT] = np.eye(N_FEAT, dtype=npmm)

    fpk = np.zeros((N_FEAT, FP_COLS), np.float32)
    fpk[0:HIDDEN, FP_WC:FP_WC + NUM_CLASSES] = np.asarray(Wc, np.float32)
    fpk[0:HIDDEN, FP_B1] = np.asarray(b1, np.float32)
    fpk[0:HIDDEN, FP_B2] = np.asarray(b2, np.float32)
    fpk[0:HIDDEN, FP_BP] = np.asarray(bp, np.float32)
    fpk[0:GPD, FP_BC:FP_BC + NUM_CLASSES] = np.asarray(bc, np.float32)[None, :]
    fpk[0:CLUSTERS, FP_ID25:FP_ID25 + CLUSTERS] = np.eye(CLUSTERS, dtype=np.float32)
    fpk[:, FP_EBA:FP_EBA + CLUSTERS] = np.exp(np.asarray(ba, np.float32))[None, :]

    common = dict(wpk=wpk, fpk=fpk)

    in_maps = []
    for d in range(DEV):
        xd = x[d * GPD * NPG:(d + 1) * GPD * NPG]          # [1200, 128]
        xT = np.ascontiguousarray(xd.T).reshape(N_FEAT, GPD, NPG).astype(npmm)
        bd = blocks[d * GPD:(d + 1) * GPD]                  # [8, 150, 150]
        bt = np.ascontiguousarray(bd.transpose(1, 0, 2))    # [150, 8, 150]
        in_maps.append(dict(
            xT=xT,
            ah0=np.ascontiguousarray(bt[:C0]),
            ah1=np.ascontiguousarray(bt[C0:]),
            **common,
        ))
    return in_maps


def kernel(x, a, seg_ids, num_graphs, W1, b1, W2, b2, Wa, ba, Wp, bp, Wc, bc,
           trace=False):
    if "nc" not in _CACHE:
        _CACHE["nc"] = build_nc()
    nc = _CACHE["nc"]
    in_maps = make_in_maps(x, a, W1, b1, W2, b2, Wa, ba, Wp, bp, Wc, bc)
    res = run_bass_kernel_spmd(nc, in_maps, core_ids=list(range(DEV)), trace=trace)
    logits = np.concatenate([r["out"] for r in res.results], axis=0)
    if trace:
        return logits, res
    return logits
